# revision 1
# baseline (speedup 1.0000x reference)
"""Distributed Bass/Trainium2 kernel for the batch graph-Laplacian (k-NN) loss.

Problem: z [8192, 512] fp32.  G = z z^T, d2_ij = ||z_i - z_j||^2, take the
k=10 nearest neighbours per row (self excluded), symmetrize the one-hot
adjacency W = max(A, A^T) and return
    loss = (sum_i deg_i ||z_i||^2 - sum_ij W_ij G_ij) / n.

Mathematical identity used here: the loss is exactly the sum of squared
distances over the edges of the symmetrized k-NN graph divided by n:
    loss = (S_dir - 0.5 * S_mut) / n
where S_dir  = sum over all directed top-k edges (i,j) of d2_ij,
      S_mut  = sum over directed edges that are *mutual* (i in topk(j)) of d2_ij.

Device work (8 NeuronCores, rows of z sharded 1024/core):
  Each core computes its [1024, 8192] block of nval_ij = G_ij - sq_j/2 with
  bf16 matmuls (fp32 PSUM accumulation; the -sq_j/2 rank-1 term is folded in
  as an extra K=2 matmul with a hi/lo bf16 split of -sq/2), then extracts the
  top-8 (value, index) per 1024-wide column chunk with the DVE Max/MaxIndex
  instructions -> 64 candidate neighbours per row.  Ordering by nval within a
  row equals ordering by d2 (d2_ij = sq_i - 2*nval_ij).
Host work (cheap glue): refine the per-row top-16 candidates with exact fp32
  dot products, pick the true top-10, resolve mutual edges on the 81920-entry
  sparse index list, and reduce the scalar.
"""

import numpy as np
import ml_dtypes

B = 8192
D = 512
K = 10
N_CORES = 8
RPC = B // N_CORES          # rows per core = 1024
RT = RPC // 128             # row tiles per core = 8
NCH = B // 512              # psum column chunks = 16
PAIR = False                # pre-reduce column pairs on the DVE (measured slower)
PAIR_GP = False             # pre-reduce pairs on GPSIMD: NOT POSSIBLE on TRN2
                            # (walrus: TensorTensor opcode illegal on Pool engine)
EW = 2048                   # extraction chunk width (max/max_index op width)
if PAIR_GP or PAIR:
    ECH = 4                 # chunks over the 4096-wide pair array
else:
    ECH = B // EW
NCAND = ECH * 8             # candidates (pairs if PAIR*) per row
REFINE = min(16, NCAND)     # candidates refined exactly on host per row

_CACHE = {}


NVAL_BF16 = True            # keep the distance-metric tile in bf16
FP8 = False                 # fp8e4m3 + DoubleRow for the main matmuls
                            # (measured slower on HW: LDWEIGHTS-bound)


def _build_program(loop_iters=None):
    """Build the SPMD Bass program.  loop_iters wraps the compute body in a
    device-side For loop (used only for wall-clock slope timing)."""
    import concourse.bacc as bacc
    import concourse.mybir as mybir
    from concourse.tile import TileContext

    dt = mybir.dt
    nval_dt = dt.bfloat16 if NVAL_BF16 else dt.float32
    nc = bacc.Bacc("TRN2", target_bir_lowering=False, debug=False,
                   num_devices=N_CORES)

    if FP8:
        # DoubleRow layout: contraction chunk c (of 2) holds z-dims
        # [256c, 256c+256) as [partition p, interleave slot i] = dim 256c+128i+p
        zq = nc.dram_tensor("zq", [2, 128, 2 * RPC], dt.float8e4,
                            kind="ExternalInput")
        zk = nc.dram_tensor("zk", [2, 128, 2 * B], dt.float8e4,
                            kind="ExternalInput")
    else:
        zq = nc.dram_tensor("zq", [D, RPC], dt.bfloat16, kind="ExternalInput")
        zk = nc.dram_tensor("zk", [D, B], dt.bfloat16, kind="ExternalInput")
    msq = nc.dram_tensor("msq", [2, B], dt.bfloat16, kind="ExternalInput")
    ones2 = nc.dram_tensor("ones2", [2, 128], dt.bfloat16, kind="ExternalInput")
    cand_val = nc.dram_tensor("cand_val", [RPC, NCAND], nval_dt,
                              kind="ExternalOutput")
    cand_idx = nc.dram_tensor("cand_idx", [RPC, NCAND], dt.uint32,
                              kind="ExternalOutput")

    with TileContext(nc) as tc:
        with (
            tc.tile_pool(name="const", bufs=1) as cpool,
            tc.tile_pool(name="nval", bufs=4) as npool,
            tc.tile_pool(name="outs", bufs=2) as opool,
            tc.tile_pool(name="psum", bufs=8, space="PSUM") as ppool,
        ):
            # Resident SBUF copies of the operands.  Load the first column
            # pieces first so the first row-tile's matmuls can start early.
            nchunk = 2 if FP8 else 4
            zdt = dt.float8e4 if FP8 else dt.bfloat16
            kwid = 2 * B if FP8 else B
            qwid = 2 * RPC if FP8 else RPC
            zk_sb = [cpool.tile([128, kwid], zdt, tag=f"zk{kc}",
                                name=f"zk_sb{kc}") for kc in range(nchunk)]
            zq_sb = [cpool.tile([128, qwid], zdt, tag=f"zq{kc}",
                                name=f"zq_sb{kc}") for kc in range(nchunk)]
            msq_sb = cpool.tile([2, B], dt.bfloat16, tag="msq")
            ones_sb = cpool.tile([2, 128], dt.bfloat16, tag="ones2")
            nc.sync.dma_start(msq_sb[:], msq[:])
            nc.sync.dma_start(ones_sb[:], ones2[:])
            if FP8:
                for kc in range(nchunk):
                    nc.sync.dma_start(zq_sb[kc][:], zq[kc, :, :])
                # order pieces so both interleave slots' leading columns land
                # first (matmul n needs columns of slot 0 AND slot 1)
                pieces = [slice(0, B // 2), slice(B, 3 * B // 2),
                          slice(B // 2, B), slice(3 * B // 2, 2 * B)]
                for sl in pieces:
                    for kc in range(nchunk):
                        nc.sync.dma_start(zk_sb[kc][:, sl], zk[kc, :, sl])
            else:
                # the first matmuls need only zq cols [0:128] and zk cols
                # [0:512]; land those first so PE starts ~10us earlier
                for sl in (slice(0, 128), slice(128, RPC)):
                    for kc in range(nchunk):
                        nc.sync.dma_start(zq_sb[kc][:, sl],
                                          zq[kc * 128:(kc + 1) * 128, sl])
                pieces = [slice(0, 512), slice(512, 2048), slice(2048, 4096),
                          slice(4096, 6144), slice(6144, B)]
                for sl in pieces:
                    for kc in range(nchunk):
                        nc.sync.dma_start(zk_sb[kc][:, sl],
                                          zk[kc * 128:(kc + 1) * 128, sl])

            from contextlib import nullcontext
            loop_cm = tc.For_i(0, loop_iters, 1) if loop_iters else nullcontext()
            with loop_cm:
                _body(nc, tc, npool, opool, ppool, zq_sb, zk_sb, msq_sb,
                      ones_sb, cand_val, cand_idx, nval_dt)

    nc.compile()
    return nc


def _body(nc, tc, npool, opool, ppool, zq_sb, zk_sb, msq_sb, ones_sb,
          cand_val, cand_idx, nval_dt):
    import concourse.mybir as mybir
    dt = mybir.dt
    if True:
            for m in range(RT):
                nval = npool.tile([128, B], nval_dt, tag="nval")
                for n in range(NCH):
                    ps = ppool.tile([128, 512], dt.float32, tag="ps")
                    csl = slice(n * 512, (n + 1) * 512)
                    if FP8:
                        for kc in range(2):
                            q3 = zq_sb[kc][:].rearrange(
                                "p (two m) -> p two m", two=2)
                            k3 = zk_sb[kc][:].rearrange(
                                "p (two n) -> p two n", two=2)
                            nc.tensor.matmul(
                                ps[:],
                                lhsT=q3[:, :, m * 128:(m + 1) * 128],
                                rhs=k3[:, :, csl],
                                start=(kc == 0),
                                stop=False,
                                perf_mode=mybir.MatmulPerfMode.DoubleRow,
                            )
                    else:
                        for kc in range(4):
                            nc.tensor.matmul(
                                ps[:],
                                lhsT=zq_sb[kc][:, m * 128:(m + 1) * 128],
                                rhs=zk_sb[kc][:, csl],
                                start=(kc == 0),
                                stop=False,
                            )
                    nc.tensor.matmul(ps[:], lhsT=ones_sb[:], rhs=msq_sb[:, csl],
                                     start=False, stop=True)
                    nc.scalar.copy(nval[:, csl], ps[:])
                vals = opool.tile([128, NCAND], nval_dt, tag="vals")
                idxs = opool.tile([128, NCAND], dt.uint32, tag="idxs")
                if PAIR_GP:
                    # pair col j with col j+B/2 on the (otherwise idle)
                    # GPSIMD engine; DVE scans the halved array
                    nvp = npool.tile([128, B // 2], nval_dt, tag="nvp")
                    for g in range(4):
                        dst = slice(g * 1024, (g + 1) * 1024)
                        hi_ = slice(B // 2 + g * 1024, B // 2 + (g + 1) * 1024)
                        nc.gpsimd.tensor_max(nvp[:, dst], nval[:, dst],
                                             nval[:, hi_])
                    for e in range(ECH):
                        esl = slice(e * 1024, (e + 1) * 1024)
                        osl = slice(e * 8, (e + 1) * 8)
                        nc.vector.max(out=vals[:, osl], in_=nvp[:, esl])
                        nc.vector.max_index(out=idxs[:, osl],
                                            in_max=vals[:, osl],
                                            in_values=nvp[:, esl])
                elif PAIR:
                    # pair col j with col j+B/2 (contiguous operand slices),
                    # then top-8 per 1024-wide chunk of the halved array
                    nvp = npool.tile([128, B // 2], nval_dt, tag="nvp")
                    for e in range(ECH):
                        dst = slice(e * 1024, (e + 1) * 1024)
                        hi_ = slice(B // 2 + e * 1024, B // 2 + (e + 1) * 1024)
                        osl = slice(e * 8, (e + 1) * 8)
                        nc.vector.tensor_max(nvp[:, dst], nval[:, dst],
                                             nval[:, hi_])
                        nc.vector.max(out=vals[:, osl], in_=nvp[:, dst])
                        nc.vector.max_index(out=idxs[:, osl],
                                            in_max=vals[:, osl],
                                            in_values=nvp[:, dst])
                else:
                    for e in range(ECH):
                        esl = slice(e * EW, (e + 1) * EW)
                        osl = slice(e * 8, (e + 1) * 8)
                        nc.vector.max(out=vals[:, osl], in_=nval[:, esl])
                        nc.vector.max_index(out=idxs[:, osl],
                                            in_max=vals[:, osl],
                                            in_values=nval[:, esl])
                rsl = slice(m * 128, (m + 1) * 128)
                nc.sync.dma_start(cand_val[rsl, :], vals[:])
                nc.sync.dma_start(cand_idx[rsl, :], idxs[:])


def _get_program():
    if "nc" not in _CACHE:
        _CACHE["nc"] = _build_program()
    return _CACHE["nc"]


def kernel(z: np.ndarray) -> np.ndarray:
    from concourse.bass_utils import run_bass_kernel_spmd

    z = np.asarray(z, dtype=np.float32)
    assert z.shape == (B, D)

    # ---- host-side prep (sharding glue) ----
    zT = np.ascontiguousarray(z.T)                       # [512, 8192] fp32
    if FP8:
        z8 = zT.astype(ml_dtypes.float8_e4m3)            # [512, 8192]
        # [chunk c, slot i, partition p, col] with dim k = 256c + 128i + p
        z4 = z8.reshape(2, 2, 128, B)
        # device layout [c, p, 2*B] slot-major in the last axis
        zk_dev = np.ascontiguousarray(
            z4.transpose(0, 2, 1, 3)).reshape(2, 128, 2 * B)
        zq_devs = [
            np.ascontiguousarray(
                z4[:, :, :, c * RPC:(c + 1) * RPC].transpose(0, 2, 1, 3)
            ).reshape(2, 128, 2 * RPC)
            for c in range(N_CORES)
        ]
    else:
        zkb = zT.astype(ml_dtypes.bfloat16)              # shared key operand
    sq = np.einsum("ij,ij->i", z.astype(np.float64), z.astype(np.float64))
    msq_f = (-0.5 * sq).astype(np.float32)               # -sq/2
    hi = msq_f.astype(ml_dtypes.bfloat16)
    lo = (msq_f - hi.astype(np.float32)).astype(ml_dtypes.bfloat16)
    msq = np.stack([hi, lo]).astype(ml_dtypes.bfloat16)  # [2, 8192]
    ones2 = np.ones((2, 128), dtype=ml_dtypes.bfloat16)

    if FP8:
        in_maps = [
            {"zq": zq_devs[c], "zk": zk_dev, "msq": msq, "ones2": ones2}
            for c in range(N_CORES)
        ]
    else:
        in_maps = [
            {
                "zq": np.ascontiguousarray(zkb[:, c * RPC:(c + 1) * RPC]),
                "zk": zkb,
                "msq": msq,
                "ones2": ones2,
            }
            for c in range(N_CORES)
        ]

    nc = _get_program()
    res = run_bass_kernel_spmd(nc, in_maps, list(range(N_CORES)))
    _CACHE["last_result"] = res

    vals = np.concatenate([res.results[c]["cand_val"] for c in range(N_CORES)])
    idxs = np.concatenate([res.results[c]["cand_idx"] for c in range(N_CORES)])

    return _postprocess(z, sq, vals, idxs)


def _postprocess(z, sq, vals, idxs):
    # decode candidate positions to global column indices
    w = 1024 if (PAIR or PAIR_GP) else EW
    pos = idxs.astype(np.int64) + (np.arange(NCAND) // 8 * w)[None, :]
    rows = np.arange(B, dtype=np.int64)
    vals = vals.astype(np.float64)

    # top-REFINE candidates by approximate metric (largest nval = smallest d2)
    part = np.argpartition(-vals, REFINE - 1, axis=1)[:, :REFINE]
    sel = np.take_along_axis(pos, part, axis=1)             # [B, REFINE]
    if PAIR or PAIR_GP:
        # each candidate is a column pair (p, p + B/2): refine both members
        cand_cols = np.concatenate([sel, sel + B // 2], axis=1)
    else:
        cand_cols = sel

    # exact squared distances for the refined candidates
    zc = z[cand_cols]
    dots = np.einsum("brd,bd->br", zc, z, optimize=True)    # fp32 accum
    d2 = sq[:, None] + sq[cand_cols] - 2.0 * dots.astype(np.float64)
    d2 = np.where(cand_cols == rows[:, None], np.inf, d2)   # drop self

    # exact top-K among the refined candidates
    sel = np.argpartition(d2, K - 1, axis=1)[:, :K]
    top_cols = np.take_along_axis(cand_cols, sel, axis=1)   # [B, 10]
    top_d2 = np.take_along_axis(d2, sel, axis=1)            # [B, 10]

    # mutual (symmetrization) correction on the sparse edge list
    edge_key = rows[:, None] * B + top_cols                 # i -> j
    rev_key = top_cols * B + rows[:, None]                  # j -> i
    mutual = np.isin(rev_key, edge_key)

    s_dir = top_d2.sum()
    s_mut = top_d2[mutual].sum()
    loss = (s_dir - 0.5 * s_mut) / B
    return np.float32(loss)

